# revision 9
# baseline (speedup 1.0000x reference)
"""Trainium2 Bass kernel for IntraFrameNet (self-attention + conv head).

Math (per sample b):
  f = curr_features[b].reshape(C, M)                      # C=128, M=4096
  S = f^T f * C^-0.5   (symmetric, [M, M])
  P = softmax(S, axis=-1)
  feats1 = f @ P^T   ([C, M]);  x = [feats1; f]           # [2C, M]
  y = W1 @ x + b1 -> BN(inference) -> leaky_relu(0.01)
  pred = w2 @ y + b2                                      # [1, M]

Design (1 sample / core, 8 cores, data-parallel):
  - S is SYMMETRIC, so exp(S) is too.  Only the lower-triangle super-blocks
    (col super b <= row super a) are exp-ed on ScalarE (10/16 of the work);
    the upper blocks are PE transposes of the lower ones (same PE cost as
    the S-matmuls they replace, but ~40% less ScalarE).
  - exp is computed with a constant input shift (softmax-invariant) so the
    unnormalized weights fit fp8e4 range; attention AV ("PV") runs as
    fp8 DoubleRow matmuls over chunk PAIRS (2 contraction slabs / pass).
  - Softmax denominators: direct tiles accumulate row sums via the ACT
    accum_out; mirrored tiles get them for free from the PSUM->SBUF copy
    (tensor_scalar with accum_out).  Together every (chunk, col-super)
    writes exactly one pD slot -> D = reduce(pD).
  - Softmax division is deferred: feats1 = O * Dinv_bcast, then
    y = W1a @ feats1 + W1b @ f (BN folded into W1a/W1b/bhead on host),
    leaky via DVE, then the w2 matmul.  Head runs per column-super as soon
    as that super's D and O are complete, overlapped with the next column.
"""

import numpy as np
import ml_dtypes

import concourse.bass as bass
from concourse import bacc
import concourse.mybir as mybir
import concourse.tile as tile
from concourse.bass_utils import run_bass_kernel_spmd
from concourse.masks import make_identity

B, C, H, W = 8, 128, 64, 64
M = H * W            # 4096
NCH = 32             # row chunks of 128
SUP = 1024           # cols per super-block
NSUP = 4
CPS = 8              # chunks per super
PPS = 4              # chunk-pairs per super
NPAIR = 16
SCALE = float(C) ** -0.5
EXP_SHIFT = -1.5     # exp(z + shift): softmax-invariant, keeps fp8 in range
BN_EPS = 1e-5
LEAKY = 0.01

PV_F8 = False         # fp8e4 + DoubleRow for the PV matmuls

f32 = mybir.dt.float32
bf16 = mybir.dt.bfloat16
f8 = mybir.dt.float8e4
PDT = f8 if PV_F8 else bf16
NP_PDT = ml_dtypes.float8_e4m3fn if PV_F8 else ml_dtypes.bfloat16
AF = mybir.ActivationFunctionType
ALU = mybir.AluOpType
DRMODE = mybir.MatmulPerfMode.DoubleRow


def _build():
    nc = bacc.Bacc("TRN2", target_bir_lowering=False)

    fb_d = nc.dram_tensor("fb", [C, M], bf16, kind="ExternalInput")
    fT_d = nc.dram_tensor("fT", [128, NCH * 128], PDT, kind="ExternalInput")
    w1aT_d = nc.dram_tensor("w1aT", [C, C], f32, kind="ExternalInput")
    w1bT_d = nc.dram_tensor("w1bT", [C, C], f32, kind="ExternalInput")
    bhead_d = nc.dram_tensor("bhead", [C, 1], f32, kind="ExternalInput")
    w2T_d = nc.dram_tensor("w2T", [C, 1], f32, kind="ExternalInput")
    sel_d = nc.dram_tensor("sel", [CPS, CPS * 128], f32, kind="ExternalInput")
    pred_d = nc.dram_tensor("pred", [1, M], f32, kind="ExternalOutput")

    with tile.TileContext(nc) as tc:
        with (
            tc.tile_pool(name="singles", bufs=1) as singles,
            tc.tile_pool(name="pbp", bufs=8) as pbp,
            tc.tile_pool(name="pbmp", bufs=4) as pbmp,
            tc.tile_pool(name="sbm", bufs=2) as sbm,
            tc.tile_pool(name="ps_st", bufs=2, space="PSUM") as ps_st,
            tc.tile_pool(name="ps_ot", bufs=1, space="PSUM") as ps_ot,
            tc.tile_pool(name="ps_tr", bufs=2, space="PSUM") as ps_tr,
        ):
            # ---------------- loads ----------------
            fb = singles.tile([C, M], bf16)
            for q in range(2):
                nc.sync.dma_start(
                    out=fb[:, q * 512 : (q + 1) * 512],
                    in_=fb_d[:, q * 512 : (q + 1) * 512],
                )
            fT = singles.tile([128, NCH, 128], PDT)
            nc.gpsimd.dma_start(out=fT[:, 0:8, :], in_=fT_d[:, 0:1024])
            w1aT_f = singles.tile([C, C], f32)
            nc.gpsimd.dma_start(out=w1aT_f, in_=w1aT_d[:, :])
            w1bT_f = singles.tile([C, C], f32)
            nc.gpsimd.dma_start(out=w1bT_f, in_=w1bT_d[:, :])
            bhead = singles.tile([C, 1], f32)
            nc.gpsimd.dma_start(out=bhead, in_=bhead_d[:, :])
            w2T_f = singles.tile([C, 1], f32)
            nc.gpsimd.dma_start(out=w2T_f, in_=w2T_d[:, :])
            sel_f = singles.tile([CPS, CPS * 128], f32)
            nc.gpsimd.dma_start(out=sel_f, in_=sel_d[:, :])
            for q in range(2, 8):
                nc.sync.dma_start(
                    out=fb[:, q * 512 : (q + 1) * 512],
                    in_=fb_d[:, q * 512 : (q + 1) * 512],
                )
            for p in range(1, 4):
                nc.gpsimd.dma_start(
                    out=fT[:, p * 8 : (p + 1) * 8, :],
                    in_=fT_d[:, p * 1024 : (p + 1) * 1024],
                )

            # identities (gpsimd) + small converts (DVE)
            ident_p = singles.tile([128, 128], PDT)
            make_identity(nc, ident_p)
            ident_f32 = singles.tile([128, 128], f32)
            make_identity(nc, ident_f32)
            w1aT_b = singles.tile([C, C], bf16)
            nc.vector.tensor_copy(out=w1aT_b, in_=w1aT_f)
            w1bT_b = singles.tile([C, C], bf16)
            nc.vector.tensor_copy(out=w1bT_b, in_=w1bT_f)
            w2T_b = singles.tile([C, 1], bf16)
            nc.vector.tensor_copy(out=w2T_b, in_=w2T_f)
            sel_b = singles.tile([CPS, CPS * 128], bf16)
            nc.vector.tensor_copy(out=sel_b, in_=sel_f)

            pD = singles.tile([128, NCH, NSUP], f32)
            pred_sb = singles.tile([1, M], f32)
            shift_ap = singles.tile([128, 1], f32)
            nc.gpsimd.memset(shift_ap, EXP_SHIFT)

            # ---------------- helpers ----------------
            pb_tiles = {}     # u -> current-column direct pair tile
            pbm_tiles = {}    # (v, col) -> mirror pair tile
            pending = []      # deferred mirror jobs
            ot_tiles = {}     # col -> PSUM accumulation tile
            pv_count = [0]

            def emit_s_exp(u, b, q):
                """S matmuls + exp for chunk t = 2u+q of column b."""
                t = 2 * u + q
                st = ps_st.tile([128, SUP], f32, tag="st", name=f"st{b}_{t}")
                for h in range(2):
                    nc.tensor.matmul(
                        st[:, h * 512 : (h + 1) * 512],
                        lhsT=fb[:, t * 128 : (t + 1) * 128],
                        rhs=fb[:, b * SUP + h * 512 : b * SUP + (h + 1) * 512],
                        start=True,
                        stop=True,
                    )
                if q == 0:
                    pb_tiles[u] = pbp.tile(
                        [128, 2, SUP], PDT, tag="pb", name=f"pb{b}_{u}"
                    )
                nc.scalar.activation(
                    out=pb_tiles[u][:, q, :],
                    in_=st,
                    func=AF.Exp,
                    scale=SCALE,
                    bias=shift_ap,
                    accum_out=pD[:, t, b : b + 1],
                )

            def emit_pv(u, rhs_pair, b):
                """PV contribution of chunk pair u into ot[b]."""
                ot = ot_tiles[b]
                n = pv_count[0]
                last = n == NPAIR - 1
                for h in range(2):
                    if PV_F8:
                        nc.tensor.matmul(
                            ot[:, h * 512 : (h + 1) * 512],
                            lhsT=fT[:, 2 * u : 2 * u + 2, :],
                            rhs=rhs_pair[:, :, h * 512 : (h + 1) * 512],
                            start=(n == 0),
                            stop=last,
                            perf_mode=DRMODE,
                        )
                    else:
                        for q in range(2):
                            nc.tensor.matmul(
                                ot[:, h * 512 : (h + 1) * 512],
                                lhsT=fT[:, 2 * u + q, :],
                                rhs=rhs_pair[:, q, h * 512 : (h + 1) * 512],
                                start=(n == 0 and q == 0),
                                stop=(last and q == 1),
                            )
                pv_count[0] += 1

            def emit_mirror_job(job):
                """Transpose sources into the mirror pair tile for (v, acol)."""
                v, acol, srcs, b_src, _ci = job
                pbm = pbmp.tile(
                    [128, 2, SUP], PDT, tag=f"m{b_src}_{acol}",
                    name=f"pbm{acol}_{v}",
                )
                for q in range(2):
                    tau = 2 * v + q
                    jl = tau % CPS
                    # fp8 PE transposes must write with element step 2
                    trp = ps_tr.tile(
                        [128, SUP, 2] if PV_F8 else [128, SUP],
                        PDT, tag="tr", name=f"trp{acol}_{tau}",
                    )
                    for j2 in range(CPS):
                        src = srcs[j2 // 2][:, j2 % 2, :]
                        nc.tensor.transpose(
                            trp[:, j2 * 128 : (j2 + 1) * 128, 0]
                            if PV_F8
                            else trp[:, j2 * 128 : (j2 + 1) * 128],
                            src[:, jl * 128 : (jl + 1) * 128],
                            ident_p,
                        )
                    nc.vector.tensor_scalar(
                        out=pbm[:, q, :],
                        in0=trp[:, :, 0] if PV_F8 else trp,
                        scalar1=0.0,
                        scalar2=0.0,
                        op0=ALU.add,
                        op1=ALU.add,
                        accum_out=pD[:, tau, acol : acol + 1],
                    )
                pbm_tiles[(v, acol)] = pbm

            def emit_head(g):
                """Per-super head: softmax divide + conv/BN/leaky + conv."""
                Dg = sbm.tile([128, CPS], f32, tag="Dg", name=f"Dg{g}")
                nc.vector.tensor_reduce(
                    out=Dg,
                    in_=pD[:, CPS * g : CPS * (g + 1), :],
                    axis=mybir.AxisListType.X,
                    op=ALU.add,
                )
                Dinvg = sbm.tile([128, CPS], f32, tag="Dinv", name=f"Dinv{g}")
                nc.vector.reciprocal(out=Dinvg, in_=Dg)
                drpg = ps_tr.tile([CPS, 128], f32, tag="tr", name=f"drpg{g}")
                nc.tensor.transpose(drpg, Dinvg, ident_f32)
                DrowTg = sbm.tile([CPS, 128], bf16, tag="DrowT", name=f"DrowT{g}")
                nc.vector.tensor_copy(out=DrowTg, in_=drpg)
                ot = ot_tiles.pop(g)
                for h in range(2):
                    base = g * SUP + h * 512
                    dbp = ps_st.tile([128, 512], f32, tag="st", name=f"dbp{g}_{h}")
                    for j in range(4):
                        jj = h * 4 + j
                        nc.tensor.matmul(
                            dbp[:, j * 128 : (j + 1) * 128],
                            lhsT=sel_b[:, jj * 128 : (jj + 1) * 128],
                            rhs=DrowTg,
                            start=True,
                            stop=True,
                        )
                    dinvb = sbm.tile([128, 512], f32, tag="dinvb", name=f"dinvb{g}_{h}")
                    nc.vector.tensor_copy(out=dinvb, in_=dbp)
                    fnorm = sbm.tile([128, 512], bf16, tag="fnorm", name=f"fnorm{g}_{h}")
                    nc.vector.tensor_tensor(
                        out=fnorm,
                        in0=ot[:, h * 512 : (h + 1) * 512],
                        in1=dinvb,
                        op=ALU.mult,
                    )
                    yp = ps_st.tile([128, 512], f32, tag="st", name=f"yp{g}_{h}")
                    nc.tensor.matmul(yp, lhsT=w1aT_b, rhs=fnorm, start=True, stop=False)
                    nc.tensor.matmul(
                        yp,
                        lhsT=w1bT_b,
                        rhs=fb[:, base : base + 512],
                        start=False,
                        stop=True,
                    )
                    t1 = sbm.tile([128, 512], f32, tag="t1", name=f"t1_{g}_{h}")
                    nc.vector.tensor_scalar_add(out=t1, in0=yp, scalar1=bhead)
                    zsb = sbm.tile([128, 512], bf16, tag="zsb", name=f"zsb{g}_{h}")
                    nc.vector.scalar_tensor_tensor(
                        out=zsb, in0=t1, scalar=LEAKY, in1=t1,
                        op0=ALU.mult, op1=ALU.max,
                    )
                    pp = ps_st.tile([1, 512], f32, tag="st", name=f"pp{g}_{h}")
                    nc.tensor.matmul(pp, lhsT=w2T_b, rhs=zsb, start=True, stop=True)
                    nc.vector.tensor_copy(out=pred_sb[0:1, base : base + 512], in_=pp)

            # ---------------- main loop ----------------
            # Per column-super b: direct subgroups processed in DESCENDING
            # super order (3, 2, .., b) so that mirror-transpose jobs are
            # created early and drain during the column (never a tail burst).
            for b in range(NSUP):
                pv_count[0] = 0
                pairs = [
                    a * PPS + k
                    for a in range(NSUP - 1, b - 1, -1)
                    for k in range(PPS)
                ]
                mir = [(v, pbm_tiles.pop((v, b))) for v in range(PPS * b)]
                nd, nm = len(pairs), len(mir)
                mi = 0
                deferred_pv = []
                for i, u in enumerate(pairs):
                    a_sup = u // PPS
                    for q in range(2):
                        emit_s_exp(u, b, q)
                    if b > 0 and i < 2:
                        # keep ACT fed before the previous head's PE/DVE
                        # chain; ot[b] doesn't exist yet (WAR on ot[b-1])
                        deferred_pv.append(u)
                    else:
                        if b > 0 and i == 2:
                            emit_head(b - 1)
                        if i == (2 if b > 0 else 0):
                            ot_tiles[b] = ps_ot.tile(
                                [C, SUP], f32, tag="ot", name=f"ot{b}"
                            )
                            for du in deferred_pv:
                                emit_pv(du, pb_tiles[du], b)
                        emit_pv(u, pb_tiles[u], b)
                    # share of this column's mirror-PV contributions
                    if b in ot_tiles:
                        want = ((i + 1) * nm) // nd
                        while mi < want:
                            v, pbm = mir[mi]
                            emit_pv(v, pbm, b)
                            mi += 1
                    # subgroup complete -> queue mirror jobs for column a_sup
                    if u % PPS == PPS - 1 and a_sup > b:
                        srcs = [pb_tiles[a_sup * PPS + k] for k in range(PPS)]
                        for v in range(PPS * b, PPS * b + PPS):
                            pending.append((v, a_sup, srcs, b, i))
                    # drain transpose jobs created at least one pair ago
                    for _ in range(2):
                        if pending and pending[0][4] < i:
                            emit_mirror_job(pending.pop(0))
                assert not pending, f"col {b}: undrained mirror jobs"
            emit_head(NSUP - 1)

            nc.sync.dma_start(out=pred_d[:, :], in_=pred_sb)

    nc.finalize()
    return nc


_NC = None


def _get_nc():
    global _NC
    if _NC is None:
        _NC = _build()
    return _NC


def _prep_host(inputs):
    curr = np.asarray(inputs["curr_features"], np.float32)
    w1 = np.asarray(inputs["w1"], np.float32)
    b1 = np.asarray(inputs["b1"], np.float32)
    gamma = np.asarray(inputs["gamma"], np.float32)
    beta = np.asarray(inputs["beta"], np.float32)
    rm = np.asarray(inputs["running_mean"], np.float32)
    rv = np.asarray(inputs["running_var"], np.float32)
    w2 = np.asarray(inputs["w2"], np.float32)

    # fold BN (inference) into the first conv
    a = gamma / np.sqrt(rv + BN_EPS)
    W1f = w1 * a[:, None]
    bhead = (b1 * a + beta - rm * a).astype(np.float32).reshape(C, 1)
    w1aT = np.ascontiguousarray(W1f[:, :C].T, np.float32)
    w1bT = np.ascontiguousarray(W1f[:, C:].T, np.float32)
    w2T = np.ascontiguousarray(w2.T, np.float32)

    selm = np.zeros((CPS, CPS * 128), np.float32)
    for k in range(CPS):
        selm[k, k * 128 : (k + 1) * 128] = 1.0

    in_maps = []
    for s in range(B):
        f = np.ascontiguousarray(curr[s].reshape(C, M))
        fbh = f.astype(ml_dtypes.bfloat16)
        fTh = np.ascontiguousarray(
            f.T.reshape(NCH, 128, C).transpose(1, 0, 2)
        ).astype(NP_PDT).reshape(128, NCH * 128)
        in_maps.append(
            {
                "fb": fbh,
                "fT": np.ascontiguousarray(fTh),
                "w1aT": w1aT,
                "w1bT": w1bT,
                "bhead": bhead,
                "w2T": w2T,
                "sel": selm,
            }
        )
    return in_maps


def kernel(**inputs):
    b2 = np.asarray(inputs["b2"], np.float32)
    in_maps = _prep_host(inputs)
    nc = _get_nc()
    res = run_bass_kernel_spmd(nc, in_maps, core_ids=list(range(B)))
    preds = np.stack([r["pred"].reshape(1, H, W) for r in res.results], axis=0)
    return (preds + b2[0]).astype(np.float32)


if __name__ == "__main__":
    _build()
    print("build OK")
